# revision 10
# baseline (speedup 1.0000x reference)
"""BiRGAT Trainium2 kernel: 8-core dst-sharded GNN message passing.

Strategy: shard edges by destination-node range (2500 dst rows per core, both
node types). Node-feature matmuls (xl tables) are replicated per core; per-edge
work is fully local (segment softmax + aggregation via one-hot sel matmuls on
sorted edges); layer outputs are AllGathered between layers. Gathers of
xl[src] rows use dma_gather (SWDGE) at window granularity.
"""
import sys, types
sys.path.insert(0, "/opt/trn_rl_repo")

import numpy as np
import ml_dtypes
import concourse.bass as bass
from concourse import bacc
import concourse.mybir as mybir
import concourse.tile as tile
from concourse.bass import ds
from concourse.bass_utils import run_bass_kernel_spmd
from concourse.masks import make_identity

F32 = mybir.dt.float32
AF = mybir.ActivationFunctionType

NCORES = 8
H, C = 4, 64
HC = H * C          # 256
PROJ = 256
CLASSES = 5
HID = 128
TYPES = ('m', 'i')
RELS = {'mm': ('m', 'm'), 'mi': ('m', 'i'), 'im': ('i', 'm'), 'ii': ('i', 'i')}
W_DST = 125          # dst rows per window


def build_program(N, T):
    """One SPMD program. N = nodes per type, T = edge tiles per window.
    Per core: NLOC = N//8 dst rows per type, NW = NLOC//W_DST windows."""
    NLOC = N // NCORES
    NW = NLOC // W_DST
    NI = T * 128                 # padded edges per window
    NTILE_FULL = N // W_DST      # full-table node tiles

    nc = bacc.Bacc(num_swdge_queues=2)

    # ---------------- I/O ----------------
    xT0 = {t: nc.dram_tensor(f"xT0_{t}", [PROJ, N], F32, kind="ExternalInput")
           for t in TYPES}
    xT0own = {t: nc.dram_tensor(f"xT0own_{t}", [PROJ, NLOC], F32, kind="ExternalInput")
              for t in TYPES}
    # per-relation edge data (per-core values)
    gidx = {r: nc.dram_tensor(f"gidx_{r}", [NW * 128, NI // 16], mybir.dt.int16,
                              kind="ExternalInput") for r in RELS}
    dcol = {r: nc.dram_tensor(f"dcol_{r}", [NW * 128, T], F32, kind="ExternalInput")
            for r in RELS}
    # weights per layer/relation
    Wl = {}; Wr = {}; attb = {}; brx = {}
    for li in (1, 2, 3):
        for r in RELS:
            Wl[li, r] = nc.dram_tensor(f"Wl{li}{r}", [HC, HC], F32, kind="ExternalInput")
            Wr[li, r] = nc.dram_tensor(f"Wr{li}{r}", [HC, HC], F32, kind="ExternalInput")
            attb[li, r] = nc.dram_tensor(f"att{li}{r}", [HC, H], F32, kind="ExternalInput")
            brx[li, r] = nc.dram_tensor(f"brx{li}{r}", [1, HC], F32, kind="ExternalInput")
    Wsl = {}; ebias = {}
    for li in (1, 2, 3):
        for t in TYPES:
            Wsl[li, t] = nc.dram_tensor(f"Wsl{li}{t}", [HC, C], F32, kind="ExternalInput")
            eb_w = HC if li < 3 else C
            ebias[li, t] = nc.dram_tensor(f"eb{li}{t}", [1, eb_w], F32, kind="ExternalInput")
    watt = nc.dram_tensor("watt", [1, C], F32, kind="ExternalInput")
    W1 = nc.dram_tensor("W1", [C, HID], F32, kind="ExternalInput")
    b1 = nc.dram_tensor("b1", [1, HID], F32, kind="ExternalInput")
    W2 = nc.dram_tensor("W2", [HID, CLASSES], F32, kind="ExternalInput")
    b2 = nc.dram_tensor("b2", [1, CLASSES], F32, kind="ExternalInput")
    out = nc.dram_tensor("out", [NLOC * 2, CLASSES], F32, kind="ExternalOutput")

    # ---------------- internal DRAM ----------------
    xl = {r: nc.dram_tensor(f"xl_{r}", [N, HC], F32) for r in RELS}       # per layer reuse
    xr = {r: nc.dram_tensor(f"xr_{r}", [NLOC, HC], F32) for r in RELS}
    sl = {t: nc.dram_tensor(f"sl_{t}", [NLOC, C], F32) for t in TYPES}
    shard = {t: nc.dram_tensor(f"shard_{t}", [NLOC, HC], F32) for t in TYPES}
    xfull = {t: nc.dram_tensor(f"xfull_{t}", [N, HC], F32, addr_space="Shared")
             for t in TYPES}
    x3 = {t: nc.dram_tensor(f"x3_{t}", [NLOC, C], F32) for t in TYPES}

    rels_by_src = {t: [r for r, (s, d) in RELS.items() if s == t] for t in TYPES}
    rels_by_dst = {t: [r for r, (s, d) in RELS.items() if d == t] for t in TYPES}
    RG = [list(range(NCORES))]

    from contextlib import ExitStack
    _es = ExitStack()
    with tile.TileContext(nc) as tc:
        cpool = _es.enter_context(tc.tile_pool(name="const", bufs=1))
        ident = cpool.tile([128, 128], F32)
        make_identity(nc, ident[:])
        iota_i = cpool.tile([128, W_DST], mybir.dt.int32)
        nc.gpsimd.iota(iota_i[:], pattern=[[1, W_DST]], base=0, channel_multiplier=0)
        iota = cpool.tile([128, W_DST], F32)
        nc.vector.tensor_copy(iota[:], iota_i[:])
        idW = ident[0:W_DST, 0:W_DST]
        ones1 = cpool.tile([1, 128], F32)
        nc.vector.memset(ones1[:], 1.0)

        # weight tiles resident in SBUF
        wpool = _es.enter_context(tc.tile_pool(name="weights", bufs=1))
        Wl_s = {}; Wr_s = {}; attb_s = {}; brx_s = {}; Wsl_s = {}; eb_s = {}
        for li in (1, 2, 3):
            for r in RELS:
                for si, (dram, store) in enumerate(((Wl[li, r], Wl_s), (Wr[li, r], Wr_s))):
                    tl = wpool.tile([128, 2, HC], F32, tag=f"w{li}{r}{si}")
                    nc.sync.dma_start(tl[:, 0, :], dram[0:128, :])
                    nc.sync.dma_start(tl[:, 1, :], dram[128:256, :])
                    store[li, r] = tl
                a = wpool.tile([128, 2, H], F32, tag=f"a{li}{r}")
                nc.sync.dma_start(a[:, 0, :], attb[li, r][0:128, :])
                nc.sync.dma_start(a[:, 1, :], attb[li, r][128:256, :])
                attb_s[li, r] = a
                b = wpool.tile([1, HC], F32, tag=f"b{li}{r}")
                nc.sync.dma_start(b[:], brx[li, r][:])
                brx_s[li, r] = b
            for t in TYPES:
                wt = wpool.tile([128, 2, C], F32, tag=f"wsl{li}{t}")
                nc.sync.dma_start(wt[:, 0, :], Wsl[li, t][0:128, :])
                nc.sync.dma_start(wt[:, 1, :], Wsl[li, t][128:256, :])
                Wsl_s[li, t] = wt
                eb_w = HC if li < 3 else C
                e = wpool.tile([W_DST, eb_w], F32, tag=f"eb{li}{t}")
                nc.sync.dma_start(e[:], ebias[li, t][:].to_broadcast([W_DST, eb_w]))
                eb_s[li, t] = e
        watt_s = wpool.tile([W_DST, C], F32)
        nc.sync.dma_start(watt_s[:], watt[:].to_broadcast([W_DST, C]))
        W1_s = wpool.tile([C, HID], F32); nc.sync.dma_start(W1_s[:], W1[:])
        b1_s = wpool.tile([1, HID], F32); nc.sync.dma_start(b1_s[:], b1[:])
        W2_s = wpool.tile([HID, CLASSES], F32); nc.sync.dma_start(W2_s[:], W2[:])
        b2_s = wpool.tile([1, CLASSES], F32); nc.sync.dma_start(b2_s[:], b2[:])

        np_pool = _es.enter_context(tc.tile_pool(name="node", bufs=2))
        ed_pool = _es.enter_context(tc.tile_pool(name="edge", bufs=3))
        sm_pool = _es.enter_context(tc.tile_pool(name="small", bufs=3))

        def node_full(li, t, src_dram):
            """xl tables for rels with src type t over all N rows.
            src_dram: None for layer1 (use xT0), else xfull[t]."""
            ps_ctx = tc.tile_pool(name="psn", bufs=2, space="PSUM")
            ps_pool = ps_ctx.__enter__()
            def body(i):
                if src_dram is None:
                    l0 = np_pool.tile([128, W_DST], F32, tag="l0")
                    l1 = np_pool.tile([128, W_DST], F32, tag="l1")
                    nc.sync.dma_start(l0[:], xT0[t][0:128, ds(i * W_DST, W_DST)])
                    nc.sync.dma_start(l1[:], xT0[t][128:256, ds(i * W_DST, W_DST)])
                else:
                    xin = np_pool.tile([W_DST, HC], F32, tag="xin")
                    nc.sync.dma_start(xin[:], src_dram[ds(i * W_DST, W_DST), :])
                    l0 = np_pool.tile([128, W_DST], F32, tag="l0")
                    l1 = np_pool.tile([128, W_DST], F32, tag="l1")
                    for h, lt in ((0, l0), (1, l1)):
                        pt = ps_pool.tile([128, W_DST], F32, space="PSUM", tag="ptr")
                        nc.tensor.transpose(pt[:], xin[:, h * 128:(h + 1) * 128], idW)
                        nc.scalar.copy(lt[:], pt[:])
                for r in rels_by_src[t]:
                    ps = ps_pool.tile([W_DST, HC], F32, space="PSUM", tag="pnode")
                    nc.tensor.matmul(ps[:], lhsT=l0[:], rhs=Wl_s[li, r][:, 0, :],
                                     start=True, stop=False)
                    nc.tensor.matmul(ps[:], lhsT=l1[:], rhs=Wl_s[li, r][:, 1, :],
                                     start=False, stop=True)
                    xs = np_pool.tile([W_DST, HC], F32, tag="xs")
                    nc.scalar.copy(xs[:], ps[:])
                    nc.sync.dma_start(xl[r][ds(i * W_DST, W_DST), :], xs[:])
            tc.For_i_unrolled(0, NTILE_FULL, 1, body, max_unroll=8)
            ps_ctx.__exit__(None, None, None)

        def node_own(li, t, src_dram):
            """xr tables (rels with dst type t) + selfloop, own rows only.
            src_dram: None for layer1 (xT0own), else shard[t] (local copy)."""
            ps_ctx = tc.tile_pool(name="pso", bufs=2, space="PSUM")
            ps_pool = ps_ctx.__enter__()
            def body(i):
                if src_dram is None:
                    l0 = np_pool.tile([128, W_DST], F32, tag="l0")
                    l1 = np_pool.tile([128, W_DST], F32, tag="l1")
                    nc.sync.dma_start(l0[:], xT0own[t][0:128, ds(i * W_DST, W_DST)])
                    nc.sync.dma_start(l1[:], xT0own[t][128:256, ds(i * W_DST, W_DST)])
                else:
                    xin = np_pool.tile([W_DST, HC], F32, tag="xin")
                    nc.sync.dma_start(xin[:], src_dram[ds(i * W_DST, W_DST), :])
                    l0 = np_pool.tile([128, W_DST], F32, tag="l0")
                    l1 = np_pool.tile([128, W_DST], F32, tag="l1")
                    for h, lt in ((0, l0), (1, l1)):
                        pt = ps_pool.tile([128, W_DST], F32, space="PSUM", tag="ptr")
                        nc.tensor.transpose(pt[:], xin[:, h * 128:(h + 1) * 128], idW)
                        nc.scalar.copy(lt[:], pt[:])
                for r in rels_by_dst[t]:
                    ps = ps_pool.tile([W_DST, HC], F32, space="PSUM", tag="pnode")
                    nc.tensor.matmul(ps[:], lhsT=l0[:], rhs=Wr_s[li, r][:, 0, :],
                                     start=True, stop=False)
                    nc.tensor.matmul(ps[:], lhsT=l1[:], rhs=Wr_s[li, r][:, 1, :],
                                     start=False, stop=False)
                    nc.tensor.matmul(ps[:], lhsT=ones1[:, 0:W_DST], rhs=brx_s[li, r][:],
                                     start=False, stop=True)
                    xs = np_pool.tile([W_DST, HC], F32, tag="xs")
                    nc.scalar.copy(xs[:], ps[:])
                    nc.sync.dma_start(xr[r][ds(i * W_DST, W_DST), :], xs[:])
                ps = ps_pool.tile([W_DST, C], F32, space="PSUM", tag="psl")
                nc.tensor.matmul(ps[:], lhsT=l0[:], rhs=Wsl_s[li, t][:, 0, :],
                                 start=True, stop=False)
                nc.tensor.matmul(ps[:], lhsT=l1[:], rhs=Wsl_s[li, t][:, 1, :],
                                 start=False, stop=True)
                xs = np_pool.tile([W_DST, C], F32, tag="xsl")
                nc.scalar.copy(xs[:], ps[:])
                nc.sync.dma_start(sl[t][ds(i * W_DST, W_DST), :], xs[:])
            tc.For_i_unrolled(0, NW, 1, body, max_unroll=4)
            ps_ctx.__exit__(None, None, None)

        def edge_phase(li, t):
            """For each window: two relations -> agg psum -> epilogue."""
            relA, relB = rels_by_dst[t]
            out_dram = shard[t] if li < 3 else x3[t]
            ow = HC if li < 3 else C
            pa_ctx = tc.tile_pool(name="pea", bufs=1, space="PSUM")
            pb_ctx = tc.tile_pool(name="peb", bufs=2, space="PSUM")
            pc_ctx = tc.tile_pool(name="pec", bufs=2, space="PSUM")
            pa = pa_ctx.__enter__(); pb = pb_ctx.__enter__(); pc = pc_ctx.__enter__()

            def body(w):
                agg = pa.tile([W_DST, HC], F32, space="PSUM", tag="agg")
                for ri, r in enumerate((relA, relB)):
                    it = sm_pool.tile([128, NI // 16], mybir.dt.int16, tag="it")
                    nc.sync.dma_start(it[:], gidx[r][ds(w * 128, 128), :])
                    gb = ed_pool.tile([128, T, HC], F32, tag="gb")
                    nc.gpsimd.dma_gather(out_ap=gb[:], in_ap=xl[r][:], idxs_ap=it[:],
                                         num_idxs=NI, num_idxs_reg=NI, elem_size=HC,
                                         queue_num=ri, single_packet=False)
                    dc = sm_pool.tile([128, T], F32, tag="dc")
                    nc.sync.dma_start(dc[:], dcol[r][ds(w * 128, 128), :])
                    xrw = sm_pool.tile([W_DST, HC], F32, tag="xrw")
                    nc.sync.dma_start(xrw[:], xr[r][ds(w * W_DST, W_DST), :])
                    selb = ed_pool.tile([128, T, W_DST], F32, tag="selb")
                    selTb = ed_pool.tile([W_DST, T, 128], F32, tag="selTb")
                    expb = ed_pool.tile([128, T, H], F32, tag="expb")
                    den = pa.tile([W_DST, H], F32, space="PSUM", tag="den")
                    for j in range(T):
                        nc.vector.tensor_tensor(
                            out=selb[:, j, :],
                            in0=dc[:, j:j + 1].to_broadcast([128, W_DST]),
                            in1=iota[:], op=mybir.AluOpType.is_equal)
                        pt = pa.tile([W_DST, 128], F32, space="PSUM", tag="pselT")
                        nc.tensor.transpose(pt[:], selb[:, j, :], ident[:])
                        nc.scalar.copy(selTb[:, j, :], pt[:])
                        zt = pb.tile([128, 2, 128], F32, space="PSUM", tag="zt")
                        wt = ed_pool.tile([128, 2, 128], F32, tag="wt")
                        sc = pc.tile([128, H], F32, space="PSUM", tag="sc")
                        for h in range(2):
                            nc.tensor.matmul(zt[:, h, :], lhsT=gb[:, j, ds(h * 128, 128)],
                                             rhs=ident[:], is_transpose=True,
                                             start=True, stop=False)
                            nc.tensor.matmul(zt[:, h, :], lhsT=xrw[:, ds(h * 128, 128)],
                                             rhs=selTb[:, j, :], start=False, stop=True)
                            nc.scalar.activation(wt[:, h, :], zt[:, h, :], AF.Prelu,
                                                 alpha=0.2)
                            nc.tensor.matmul(sc[:], lhsT=wt[:, h, :],
                                             rhs=attb_s[li, r][:, h, :],
                                             start=(h == 0), stop=(h == 1))
                        nc.scalar.activation(expb[:, j, :], sc[:], AF.Exp)
                        nc.tensor.matmul(den[:], lhsT=selb[:, j, :], rhs=expb[:, j, :],
                                         start=(j == 0), stop=(j == T - 1))
                    rden = sm_pool.tile([W_DST, H], F32, tag="rden")
                    dent = sm_pool.tile([W_DST, H], F32, tag="dent")
                    nc.scalar.activation(dent[:], den[:], AF.Copy, bias=1e-16)
                    nc.vector.reciprocal(rden[:], dent[:])
                    for j in range(T):
                        rex = pa.tile([128, H], F32, space="PSUM", tag="rex")
                        nc.tensor.matmul(rex[:], lhsT=selTb[:, j, :], rhs=rden[:],
                                         start=True, stop=True)
                        alp = sm_pool.tile([128, H], F32, tag="alp")
                        nc.vector.tensor_mul(alp[:], expb[:, j, :], rex[:])
                        msg = ed_pool.tile([128, H, C], F32, tag="msg")
                        nc.vector.tensor_tensor(
                            out=msg[:],
                            in0=gb[:, j, :].rearrange("p (h c) -> p h c", h=H),
                            in1=alp[:, :, None].to_broadcast([128, H, C]),
                            op=mybir.AluOpType.mult)
                        nc.tensor.matmul(agg[:], lhsT=selb[:, j, :],
                                         rhs=msg[:].rearrange("p h c -> p (h c)"),
                                         start=(ri == 0 and j == 0),
                                         stop=(ri == 1 and j == T - 1))
                # epilogue
                slw = sm_pool.tile([W_DST, C], F32, tag="slw")
                nc.sync.dma_start(slw[:], sl[t][ds(w * W_DST, W_DST), :])
                c1 = sm_pool.tile([W_DST, ow], F32, tag="c1")
                if li < 3:
                    nc.vector.tensor_add(c1[:], agg[:], eb_s[li, t][:])
                    nc.vector.tensor_tensor(
                        out=c1[:].rearrange("p (h c) -> p h c", h=H),
                        in0=c1[:].rearrange("p (h c) -> p h c", h=H),
                        in1=slw[:, None, :].to_broadcast([W_DST, H, C]),
                        op=mybir.AluOpType.add)
                else:
                    aggs = sm_pool.tile([W_DST, HC], F32, tag="aggs")
                    nc.scalar.copy(aggs[:], agg[:])
                    nc.vector.tensor_add(c1[:], aggs[:, 0:C], aggs[:, C:2 * C])
                    nc.vector.tensor_add(c1[:], c1[:], aggs[:, 2 * C:3 * C])
                    nc.vector.tensor_add(c1[:], c1[:], aggs[:, 3 * C:4 * C])
                    nc.scalar.mul(c1[:], c1[:], 0.25)
                    nc.vector.tensor_add(c1[:], c1[:], eb_s[li, t][:])
                    nc.vector.tensor_add(c1[:], c1[:], slw[:])
                neg = sm_pool.tile([W_DST, ow], F32, tag="neg")
                nc.vector.tensor_scalar(out=neg[:], in0=c1[:], scalar1=0.0,
                                        scalar2=None, op0=mybir.AluOpType.min)
                en = sm_pool.tile([W_DST, ow], F32, tag="en")
                nc.scalar.activation(en[:], neg[:], AF.Exp)
                pos = sm_pool.tile([W_DST, ow], F32, tag="pos")
                nc.vector.tensor_scalar(out=pos[:], in0=c1[:], scalar1=0.0,
                                        scalar2=None, op0=mybir.AluOpType.max)
                res = sm_pool.tile([W_DST, ow], F32, tag="res")
                nc.vector.tensor_add(res[:], pos[:], en[:])
                nc.scalar.activation(res[:], res[:], AF.Copy, bias=-1.0)
                nc.sync.dma_start(out_dram[ds(w * W_DST, W_DST), :], res[:])
            tc.For_i_unrolled(0, NW, 1, body, max_unroll=2)
            pc_ctx.__exit__(None, None, None)
            pb_ctx.__exit__(None, None, None); pa_ctx.__exit__(None, None, None)

        # ================= layers =================
        for li in (1, 2, 3):
            src = {1: {t: None for t in TYPES},
                   2: {t: xfull[t] for t in TYPES},
                   3: {t: xfull[t] for t in TYPES}}[li]
            srco = {1: {t: None for t in TYPES},
                    2: {t: shard[t] for t in TYPES},
                    3: {t: shard[t] for t in TYPES}}[li]
            for t in TYPES:
                node_full(li, t, src[t])
            for t in TYPES:
                node_own(li, t, srco[t])
            for t in TYPES:
                edge_phase(li, t)
            if li < 3:
                for t in TYPES:
                    nc.gpsimd.collective_compute(
                        "AllGather", mybir.AluOpType.bypass,
                        replica_groups=RG, ins=[shard[t][:]], outs=[xfull[t][:]])

        # ================= integrator =================
        pi_ctx = tc.tile_pool(name="pint", bufs=2, space="PSUM")
        ps_pool = pi_ctx.__enter__()
        def integ_body(i):
            xm = sm_pool.tile([W_DST, C], F32, tag="ixm")
            xi = sm_pool.tile([W_DST, C], F32, tag="ixi")
            nc.sync.dma_start(xm[:], x3['m'][ds(i * W_DST, W_DST), :])
            nc.sync.dma_start(xi[:], x3['i'][ds(i * W_DST, W_DST), :])
            sm_ = sm_pool.tile([W_DST, 1], F32, tag="ism")
            si_ = sm_pool.tile([W_DST, 1], F32, tag="isi")
            tmp = sm_pool.tile([W_DST, C], F32, tag="itmp")
            nc.vector.tensor_mul(tmp[:], xm[:], watt_s[:])
            nc.vector.reduce_sum(sm_[:], tmp[:], axis=mybir.AxisListType.X)
            nc.vector.tensor_mul(tmp[:], xi[:], watt_s[:])
            nc.vector.reduce_sum(si_[:], tmp[:], axis=mybir.AxisListType.X)
            dmi = sm_pool.tile([W_DST, 1], F32, tag="idm")
            nc.vector.tensor_sub(dmi[:], sm_[:], si_[:])
            am = sm_pool.tile([W_DST, 1], F32, tag="iam")
            ai = sm_pool.tile([W_DST, 1], F32, tag="iai")
            nc.scalar.activation(am[:], dmi[:], AF.Sigmoid)
            nc.vector.tensor_sub(dmi[:], si_[:], sm_[:])
            nc.scalar.activation(ai[:], dmi[:], AF.Sigmoid)
            fu = sm_pool.tile([W_DST, C], F32, tag="ifu")
            nc.vector.tensor_tensor(out=fu[:], in0=xm[:],
                                    in1=am[:].to_broadcast([W_DST, C]),
                                    op=mybir.AluOpType.mult)
            nc.vector.tensor_tensor(out=tmp[:], in0=xi[:],
                                    in1=ai[:].to_broadcast([W_DST, C]),
                                    op=mybir.AluOpType.mult)
            nc.vector.tensor_add(fu[:], fu[:], tmp[:])
            pt = ps_pool.tile([C, W_DST], F32, space="PSUM", tag="ipt")
            nc.tensor.transpose(pt[:], fu[:], idW)
            fT = sm_pool.tile([C, W_DST], F32, tag="ifT")
            nc.scalar.copy(fT[:], pt[:])
            hp = ps_pool.tile([W_DST, HID], F32, space="PSUM", tag="ihp")
            nc.tensor.matmul(hp[:], lhsT=fT[:], rhs=W1_s[:], start=True, stop=False)
            nc.tensor.matmul(hp[:], lhsT=ones1[:, 0:W_DST], rhs=b1_s[:],
                             start=False, stop=True)
            hs = sm_pool.tile([W_DST, HID], F32, tag="ihs")
            nc.scalar.activation(hs[:], hp[:], AF.Relu)
            pt2 = ps_pool.tile([HID, W_DST], F32, space="PSUM", tag="ipt2")
            nc.tensor.transpose(pt2[:], hs[:], idW)
            hT = sm_pool.tile([HID, W_DST], F32, tag="ihT")
            nc.scalar.copy(hT[:], pt2[:])
            op_ = ps_pool.tile([W_DST, CLASSES], F32, space="PSUM", tag="iop")
            nc.tensor.matmul(op_[:], lhsT=hT[:], rhs=W2_s[:], start=True, stop=False)
            nc.tensor.matmul(op_[:], lhsT=ones1[:, 0:W_DST], rhs=b2_s[:],
                             start=False, stop=True)
            os_ = sm_pool.tile([W_DST, CLASSES], F32, tag="ios")
            nc.scalar.copy(os_[:], op_[:])
            nc.sync.dma_start(out[ds(i * W_DST, W_DST), :], os_[:])
            os2 = sm_pool.tile([W_DST, CLASSES], F32, tag="ios2")
            nc.vector.tensor_copy(os2[:], os_[:])
            nc.sync.dma_start(out[ds(NLOC + i * W_DST, W_DST), :], os2[:])
        # NOTE: out rows [0:NLOC] = fused for m-type node ids, same values for
        # i-type ids because reference fuses types into one output per sample id
        tc.For_i_unrolled(0, NW, 1, integ_body, max_unroll=2)
        pi_ctx.__exit__(None, None, None)
        _es.close()

    nc.finalize()
    return nc


# ---------------------------------------------------------------- host side --
def _wrap_idxs(idx):
    n = idx.shape[0]
    w = idx.reshape(n // 16, 16).T.astype(np.int16)
    return np.tile(w, (8, 1))


def _prep_edges(edge, N, T_force=None):
    """edge [2, E] global. Returns per-core (gidx [NW,128,NI/16] int16,
    dcol [NW,128,T] f32) lists + T."""
    NLOC = N // NCORES
    NW = NLOC // W_DST
    src, dst = edge[0].astype(np.int64), edge[1].astype(np.int64)
    per_core = []
    maxT = 1
    for k in range(NCORES):
        m = (dst // NLOC) == k
        s, d = src[m], dst[m] - k * NLOC
        order = np.argsort(d, kind='stable')
        s, d = s[order], d[order]
        wins = []
        for w in range(NW):
            mm = (d // W_DST) == w
            sw, dw = s[mm], d[mm] % W_DST
            wins.append((sw, dw))
            maxT = max(maxT, (len(sw) + 127) // 128)
        per_core.append(wins)
    T = T_force or maxT
    NI = T * 128
    out = []
    for k in range(NCORES):
        gi = np.zeros((NW, 128, NI // 16), np.int16)
        dc = np.full((NW, 128, T), float(W_DST), np.float32)
        for w, (sw, dw) in enumerate(per_core[k]):
            n = len(sw)
            assert n <= NI, f"window overflow {n} > {NI}"
            si = np.zeros(NI, np.int64); si[:n] = sw
            gi[w] = _wrap_idxs(si)
            di = np.full(NI, float(W_DST), np.float32); di[:n] = dw
            dc[w] = di.reshape(T, 128).T
        out.append((gi.reshape(NW * 128, NI // 16), dc.reshape(NW * 128, T)))
    return out, T


def _np(x):
    return np.asarray(x, dtype=np.float32)


def kernel(x_mrna, x_mirna, params, edge_mm, edge_mi, edge_im, edge_ii,
           _N=None):
    N = _N or x_mrna.shape[0]
    NLOC = N // NCORES
    edges = {'mm': edge_mm, 'mi': edge_mi, 'im': edge_im, 'ii': edge_ii}
    prep = {}
    T = 1
    for r in RELS:
        prep[r], Tr = _prep_edges(np.asarray(edges[r]), N)
        T = max(T, Tr)
    # re-pad all to common T
    for r in RELS:
        prep[r], _ = _prep_edges(np.asarray(edges[r]), N, T_force=T)

    nc = build_program(N, T)

    x0 = {'m': _np(x_mrna), 'i': _np(x_mirna)}
    common = {}
    for t in TYPES:
        common[f"xT0_{t}"] = np.ascontiguousarray(x0[t].T)
    for li in (1, 2, 3):
        cp = params[f'conv{li}']
        slp = params[f'sl{li}']
        for r in RELS:
            p = cp[r]
            common[f"Wl{li}{r}"] = _np(p['Wl'])
            common[f"Wr{li}{r}"] = _np(p['Wr'])
            ab = np.zeros((HC, H), np.float32)
            att = _np(p['att'])
            for h in range(H):
                ab[h * C:(h + 1) * C, h] = att[h]
            common[f"att{li}{r}"] = ab
            common[f"brx{li}{r}"] = (_np(p['bl']) + _np(p['br']))[None, :]
        for t in TYPES:
            common[f"Wsl{li}{t}"] = _np(slp[t]['W'])
            rels_d = [r for r, (s, d) in RELS.items() if d == t]
            if li < 3:
                eb = sum(_np(cp[r]['bl']) + _np(cp[r]['bias']) for r in rels_d)
                eb = eb + np.tile(_np(slp[t]['b']), H)
            else:
                eb = sum(_np(cp[r]['bl']).reshape(H, C).mean(0) + _np(cp[r]['bias'])
                         for r in rels_d)
                eb = eb + _np(slp[t]['b'])
            common[f"eb{li}{t}"] = eb[None, :]
    ip = params['integ']
    common["watt"] = _np(ip['w_att'])[None, :]
    common["W1"] = _np(ip['W1']); common["b1"] = _np(ip['b1'])[None, :]
    common["W2"] = _np(ip['W2']); common["b2"] = _np(ip['b2'])[None, :]

    in_maps = []
    for k in range(NCORES):
        m = dict(common)
        for t in TYPES:
            m[f"xT0own_{t}"] = np.ascontiguousarray(
                x0[t][k * NLOC:(k + 1) * NLOC].T)
        for r in RELS:
            gi, dc = prep[r][k]
            m[f"gidx_{r}"] = gi
            m[f"dcol_{r}"] = dc
        in_maps.append(m)

    trace = bool(globals().get("TRACE"))
    if trace:
        _install_ntff_hook()
    res = run_bass_kernel_spmd(nc, in_maps, list(range(NCORES)), trace=trace)
    globals()["LAST_EXEC_NS"] = res.exec_time_ns
    # out rows per core: [NLOC m-fused, NLOC dup] -> reference output is per
    # sample id (types fused): take first NLOC rows of each core
    return np.concatenate([res.results[k]["out"][:NLOC] for k in range(NCORES)],
                          axis=0)


def _install_ntff_hook():
    import antenv
    if hasattr(antenv, "axon_hooks"):
        return
    from trn_agent_boot.trn_boot import _ntff_profile_via_ctypes
    hook = _ntff_profile_via_ctypes("/opt/axon/libaxon_pjrt.so")
    mod = types.ModuleType("antenv.axon_hooks")
    mod.get_axon_ntff_profile_hook = lambda: hook
    mod.set_axon_ntff_profile_hook = lambda h: None
    sys.modules["antenv.axon_hooks"] = mod
    antenv.axon_hooks = mod


# revision 12
# speedup vs baseline: 1.1646x; 1.1646x over previous
"""BiRGAT Trainium2 kernel: 8-core dst-sharded GNN message passing.

Strategy: shard edges by destination-node range (2500 dst rows per core, both
node types). Node-feature matmuls (xl tables) are replicated per core; per-edge
work is fully local (segment softmax + aggregation via one-hot sel matmuls on
sorted edges); layer outputs are AllGathered between layers. Gathers of
xl[src] rows use dma_gather (SWDGE) at window granularity.
"""
import sys, types
sys.path.insert(0, "/opt/trn_rl_repo")

import numpy as np
import ml_dtypes
import concourse.bass as bass
from concourse import bacc
import concourse.mybir as mybir
import concourse.tile as tile
from concourse.bass import ds
from concourse.bass_utils import run_bass_kernel_spmd
from concourse.masks import make_identity

F32 = mybir.dt.float32
AF = mybir.ActivationFunctionType

NCORES = 8
H, C = 4, 64
HC = H * C          # 256
PROJ = 256
CLASSES = 5
HID = 128
TYPES = ('m', 'i')
RELS = {'mm': ('m', 'm'), 'mi': ('m', 'i'), 'im': ('i', 'm'), 'ii': ('i', 'i')}
W_DST = 125          # dst rows per window


def build_program(N, T):
    """One SPMD program. N = nodes per type, T = edge tiles per window.
    Per core: NLOC = N//8 dst rows per type, NW = NLOC//W_DST windows."""
    NLOC = N // NCORES
    NW = NLOC // W_DST
    NI = T * 128                 # padded edges per window
    NTILE_FULL = N // W_DST      # full-table node tiles

    nc = bacc.Bacc(num_swdge_queues=2)

    # ---------------- I/O ----------------
    xT0 = {t: nc.dram_tensor(f"xT0_{t}", [PROJ, N], F32, kind="ExternalInput")
           for t in TYPES}
    xT0own = {t: nc.dram_tensor(f"xT0own_{t}", [PROJ, NLOC], F32, kind="ExternalInput")
              for t in TYPES}
    # per-relation edge data (per-core values)
    gidx = {r: nc.dram_tensor(f"gidx_{r}", [NW * 128, NI // 16], mybir.dt.int16,
                              kind="ExternalInput") for r in RELS}
    dcol = {r: nc.dram_tensor(f"dcol_{r}", [NW * 128, T], F32, kind="ExternalInput")
            for r in RELS}
    # weights per layer/relation
    Wl = {}; Wr = {}; attb = {}; brx = {}
    for li in (1, 2, 3):
        for r in RELS:
            Wl[li, r] = nc.dram_tensor(f"Wl{li}{r}", [HC, HC], F32, kind="ExternalInput")
            Wr[li, r] = nc.dram_tensor(f"Wr{li}{r}", [HC, HC], F32, kind="ExternalInput")
            attb[li, r] = nc.dram_tensor(f"att{li}{r}", [HC, H], F32, kind="ExternalInput")
            brx[li, r] = nc.dram_tensor(f"brx{li}{r}", [1, HC], F32, kind="ExternalInput")
    Wsl = {}; ebias = {}
    for li in (1, 2, 3):
        for t in TYPES:
            Wsl[li, t] = nc.dram_tensor(f"Wsl{li}{t}", [HC, C], F32, kind="ExternalInput")
            eb_w = HC if li < 3 else C
            ebias[li, t] = nc.dram_tensor(f"eb{li}{t}", [1, eb_w], F32, kind="ExternalInput")
    watt = nc.dram_tensor("watt", [1, C], F32, kind="ExternalInput")
    W1 = nc.dram_tensor("W1", [C, HID], F32, kind="ExternalInput")
    b1 = nc.dram_tensor("b1", [1, HID], F32, kind="ExternalInput")
    W2 = nc.dram_tensor("W2", [HID, CLASSES], F32, kind="ExternalInput")
    b2 = nc.dram_tensor("b2", [1, CLASSES], F32, kind="ExternalInput")
    out = nc.dram_tensor("out", [NLOC * 2, CLASSES], F32, kind="ExternalOutput")

    # ---------------- internal DRAM ----------------
    xl = {r: nc.dram_tensor(f"xl_{r}", [N, HC], F32) for r in RELS}       # per layer reuse
    xr = {r: nc.dram_tensor(f"xr_{r}", [NLOC, HC], F32) for r in RELS}
    sl = {t: nc.dram_tensor(f"sl_{t}", [NLOC, C], F32) for t in TYPES}
    shard = {t: nc.dram_tensor(f"shard_{t}", [NLOC, HC], F32) for t in TYPES}
    xfull = {t: nc.dram_tensor(f"xfull_{t}", [N, HC], F32, addr_space="Shared")
             for t in TYPES}
    x3 = {t: nc.dram_tensor(f"x3_{t}", [NLOC, C], F32) for t in TYPES}

    rels_by_src = {t: [r for r, (s, d) in RELS.items() if s == t] for t in TYPES}
    rels_by_dst = {t: [r for r, (s, d) in RELS.items() if d == t] for t in TYPES}
    RG = [list(range(NCORES))]

    from contextlib import ExitStack
    _es = ExitStack()
    with tile.TileContext(nc) as tc:
        cpool = _es.enter_context(tc.tile_pool(name="const", bufs=1))
        ident = cpool.tile([128, 128], F32)
        make_identity(nc, ident[:])
        iota_i = cpool.tile([128, W_DST], mybir.dt.int32)
        nc.gpsimd.iota(iota_i[:], pattern=[[1, W_DST]], base=0, channel_multiplier=0)
        iota = cpool.tile([128, W_DST], F32)
        nc.vector.tensor_copy(iota[:], iota_i[:])
        idW = ident[0:W_DST, 0:W_DST]
        ones1 = cpool.tile([1, 128], F32)
        nc.vector.memset(ones1[:], 1.0)

        # weight tiles resident in SBUF
        wpool = _es.enter_context(tc.tile_pool(name="weights", bufs=1))
        Wl_s = {}; Wr_s = {}; attb_s = {}; brx_s = {}; Wsl_s = {}; eb_s = {}
        for li in (1, 2, 3):
            for r in RELS:
                for si, (dram, store) in enumerate(((Wl[li, r], Wl_s), (Wr[li, r], Wr_s))):
                    tl = wpool.tile([128, 2, HC], F32, tag=f"w{li}{r}{si}")
                    nc.sync.dma_start(tl[:, 0, :], dram[0:128, :])
                    nc.sync.dma_start(tl[:, 1, :], dram[128:256, :])
                    store[li, r] = tl
                a = wpool.tile([128, 2, H], F32, tag=f"a{li}{r}")
                nc.sync.dma_start(a[:, 0, :], attb[li, r][0:128, :])
                nc.sync.dma_start(a[:, 1, :], attb[li, r][128:256, :])
                attb_s[li, r] = a
                b = wpool.tile([1, HC], F32, tag=f"b{li}{r}")
                nc.sync.dma_start(b[:], brx[li, r][:])
                brx_s[li, r] = b
            for t in TYPES:
                wt = wpool.tile([128, 2, C], F32, tag=f"wsl{li}{t}")
                nc.sync.dma_start(wt[:, 0, :], Wsl[li, t][0:128, :])
                nc.sync.dma_start(wt[:, 1, :], Wsl[li, t][128:256, :])
                Wsl_s[li, t] = wt
                eb_w = HC if li < 3 else C
                e = wpool.tile([W_DST, eb_w], F32, tag=f"eb{li}{t}")
                nc.sync.dma_start(e[:], ebias[li, t][:].to_broadcast([W_DST, eb_w]))
                eb_s[li, t] = e
        watt_s = wpool.tile([W_DST, C], F32)
        nc.sync.dma_start(watt_s[:], watt[:].to_broadcast([W_DST, C]))
        W1_s = wpool.tile([C, HID], F32); nc.sync.dma_start(W1_s[:], W1[:])
        b1_s = wpool.tile([1, HID], F32); nc.sync.dma_start(b1_s[:], b1[:])
        W2_s = wpool.tile([HID, CLASSES], F32); nc.sync.dma_start(W2_s[:], W2[:])
        b2_s = wpool.tile([1, CLASSES], F32); nc.sync.dma_start(b2_s[:], b2[:])

        np_pool = _es.enter_context(tc.tile_pool(name="node", bufs=2))
        ed_pool = _es.enter_context(tc.tile_pool(name="edge", bufs=3))
        sm_pool = _es.enter_context(tc.tile_pool(name="small", bufs=3))

        def node_full(li, t, src_dram):
            """xl tables for rels with src type t over all N rows.
            src_dram: None for layer1 (use xT0), else xfull[t]."""
            ps_ctx = tc.tile_pool(name="psn", bufs=2, space="PSUM")
            ps_pool = ps_ctx.__enter__()
            def body(i):
                if src_dram is None:
                    l0 = np_pool.tile([128, W_DST], F32, tag="l0")
                    l1 = np_pool.tile([128, W_DST], F32, tag="l1")
                    nc.sync.dma_start(l0[:], xT0[t][0:128, ds(i * W_DST, W_DST)])
                    nc.sync.dma_start(l1[:], xT0[t][128:256, ds(i * W_DST, W_DST)])
                else:
                    xin = np_pool.tile([W_DST, HC], F32, tag="xin")
                    nc.sync.dma_start(xin[:], src_dram[ds(i * W_DST, W_DST), :])
                    l0 = np_pool.tile([128, W_DST], F32, tag="l0")
                    l1 = np_pool.tile([128, W_DST], F32, tag="l1")
                    for h, lt in ((0, l0), (1, l1)):
                        pt = ps_pool.tile([128, W_DST], F32, space="PSUM", tag="ptr")
                        nc.tensor.transpose(pt[:], xin[:, h * 128:(h + 1) * 128], idW)
                        nc.scalar.copy(lt[:], pt[:])
                for r in rels_by_src[t]:
                    ps = ps_pool.tile([W_DST, HC], F32, space="PSUM", tag="pnode")
                    nc.tensor.matmul(ps[:], lhsT=l0[:], rhs=Wl_s[li, r][:, 0, :],
                                     start=True, stop=False)
                    nc.tensor.matmul(ps[:], lhsT=l1[:], rhs=Wl_s[li, r][:, 1, :],
                                     start=False, stop=True)
                    xs = np_pool.tile([W_DST, HC], F32, tag="xs")
                    nc.scalar.copy(xs[:], ps[:])
                    nc.sync.dma_start(xl[r][ds(i * W_DST, W_DST), :], xs[:])
            tc.For_i_unrolled(0, NTILE_FULL, 1, body, max_unroll=8)
            ps_ctx.__exit__(None, None, None)

        def node_own(li, t, src_dram):
            """xr tables (rels with dst type t) + selfloop, own rows only.
            src_dram: None for layer1 (xT0own), else shard[t] (local copy)."""
            ps_ctx = tc.tile_pool(name="pso", bufs=2, space="PSUM")
            ps_pool = ps_ctx.__enter__()
            def body(i):
                if src_dram is None:
                    l0 = np_pool.tile([128, W_DST], F32, tag="l0")
                    l1 = np_pool.tile([128, W_DST], F32, tag="l1")
                    nc.sync.dma_start(l0[:], xT0own[t][0:128, ds(i * W_DST, W_DST)])
                    nc.sync.dma_start(l1[:], xT0own[t][128:256, ds(i * W_DST, W_DST)])
                else:
                    xin = np_pool.tile([W_DST, HC], F32, tag="xin")
                    nc.sync.dma_start(xin[:], src_dram[ds(i * W_DST, W_DST), :])
                    l0 = np_pool.tile([128, W_DST], F32, tag="l0")
                    l1 = np_pool.tile([128, W_DST], F32, tag="l1")
                    for h, lt in ((0, l0), (1, l1)):
                        pt = ps_pool.tile([128, W_DST], F32, space="PSUM", tag="ptr")
                        nc.tensor.transpose(pt[:], xin[:, h * 128:(h + 1) * 128], idW)
                        nc.scalar.copy(lt[:], pt[:])
                for r in rels_by_dst[t]:
                    ps = ps_pool.tile([W_DST, HC], F32, space="PSUM", tag="pnode")
                    nc.tensor.matmul(ps[:], lhsT=l0[:], rhs=Wr_s[li, r][:, 0, :],
                                     start=True, stop=False)
                    nc.tensor.matmul(ps[:], lhsT=l1[:], rhs=Wr_s[li, r][:, 1, :],
                                     start=False, stop=False)
                    nc.tensor.matmul(ps[:], lhsT=ones1[:, 0:W_DST], rhs=brx_s[li, r][:],
                                     start=False, stop=True)
                    xs = np_pool.tile([W_DST, HC], F32, tag="xs")
                    nc.scalar.copy(xs[:], ps[:])
                    nc.sync.dma_start(xr[r][ds(i * W_DST, W_DST), :], xs[:])
                ps = ps_pool.tile([W_DST, C], F32, space="PSUM", tag="psl")
                nc.tensor.matmul(ps[:], lhsT=l0[:], rhs=Wsl_s[li, t][:, 0, :],
                                 start=True, stop=False)
                nc.tensor.matmul(ps[:], lhsT=l1[:], rhs=Wsl_s[li, t][:, 1, :],
                                 start=False, stop=True)
                xs = np_pool.tile([W_DST, C], F32, tag="xsl")
                nc.scalar.copy(xs[:], ps[:])
                nc.sync.dma_start(sl[t][ds(i * W_DST, W_DST), :], xs[:])
            tc.For_i_unrolled(0, NW, 1, body, max_unroll=4)
            ps_ctx.__exit__(None, None, None)

        def edge_phase(li, t):
            """For each window: two relations -> agg psum -> epilogue."""
            relA, relB = rels_by_dst[t]
            out_dram = shard[t] if li < 3 else x3[t]
            ow = HC if li < 3 else C
            pa_ctx = tc.tile_pool(name="pea", bufs=1, space="PSUM")
            pb_ctx = tc.tile_pool(name="peb", bufs=2, space="PSUM")
            pa = pa_ctx.__enter__(); pb = pb_ctx.__enter__()

            def body(w):
                agg = pa.tile([W_DST, HC], F32, space="PSUM", tag="agg")
                for ri, r in enumerate((relA, relB)):
                    it = sm_pool.tile([128, NI // 16], mybir.dt.int16, tag="it")
                    nc.sync.dma_start(it[:], gidx[r][ds(w * 128, 128), :])
                    gb = ed_pool.tile([128, T, HC], F32, tag="gb")
                    nc.gpsimd.dma_gather(out_ap=gb[:], in_ap=xl[r][:], idxs_ap=it[:],
                                         num_idxs=NI, num_idxs_reg=NI, elem_size=HC,
                                         queue_num=ri, single_packet=False)
                    dc = sm_pool.tile([128, T], F32, tag="dc")
                    nc.sync.dma_start(dc[:], dcol[r][ds(w * 128, 128), :])
                    xrw = sm_pool.tile([W_DST, HC], F32, tag="xrw")
                    nc.sync.dma_start(xrw[:], xr[r][ds(w * W_DST, W_DST), :])
                    selb = ed_pool.tile([128, T, W_DST], F32, tag="selb")
                    selTb = ed_pool.tile([W_DST, T, 128], F32, tag="selTb")
                    expb = ed_pool.tile([128, T, H], F32, tag="expb")
                    den = pa.tile([W_DST, H], F32, space="PSUM", tag="den")
                    for j in range(T):
                        nc.vector.tensor_tensor(
                            out=selb[:, j, :],
                            in0=dc[:, j:j + 1].to_broadcast([128, W_DST]),
                            in1=iota[:], op=mybir.AluOpType.is_equal)
                        pt = pb.tile([W_DST, 128], F32, space="PSUM", tag="pselT")
                        nc.tensor.transpose(pt[:], selb[:, j, :], ident[:])
                        nc.scalar.copy(selTb[:, j, :], pt[:])
                        zt = pb.tile([128, 2, 128], F32, space="PSUM", tag="zt")
                        wt = ed_pool.tile([128, 2, 128], F32, tag="wt")
                        sc = pa.tile([128, H], F32, space="PSUM", tag="sc")
                        for h in range(2):
                            nc.tensor.matmul(zt[:, h, :], lhsT=gb[:, j, ds(h * 128, 128)],
                                             rhs=ident[:], is_transpose=True,
                                             start=True, stop=False)
                            nc.tensor.matmul(zt[:, h, :], lhsT=xrw[:, ds(h * 128, 128)],
                                             rhs=selTb[:, j, :], start=False, stop=True)
                            nc.scalar.activation(wt[:, h, :], zt[:, h, :], AF.Prelu,
                                                 alpha=0.2)
                            nc.tensor.matmul(sc[:], lhsT=wt[:, h, :],
                                             rhs=attb_s[li, r][:, h, :],
                                             start=(h == 0), stop=(h == 1))
                        nc.scalar.activation(expb[:, j, :], sc[:], AF.Exp)
                        nc.tensor.matmul(den[:], lhsT=selb[:, j, :], rhs=expb[:, j, :],
                                         start=(j == 0), stop=(j == T - 1))
                    rden = sm_pool.tile([W_DST, H], F32, tag="rden")
                    dent = sm_pool.tile([W_DST, H], F32, tag="dent")
                    nc.scalar.activation(dent[:], den[:], AF.Copy, bias=1e-16)
                    nc.vector.reciprocal(rden[:], dent[:])
                    for j in range(T):
                        rex = pa.tile([128, H], F32, space="PSUM", tag="rex")
                        nc.tensor.matmul(rex[:], lhsT=selTb[:, j, :], rhs=rden[:],
                                         start=True, stop=True)
                        alp = sm_pool.tile([128, H], F32, tag="alp")
                        nc.vector.tensor_mul(alp[:], expb[:, j, :], rex[:])
                        msg = ed_pool.tile([128, H, C], F32, tag="msg")
                        nc.vector.tensor_tensor(
                            out=msg[:],
                            in0=gb[:, j, :].rearrange("p (h c) -> p h c", h=H),
                            in1=alp[:, :, None].to_broadcast([128, H, C]),
                            op=mybir.AluOpType.mult)
                        nc.tensor.matmul(agg[:], lhsT=selb[:, j, :],
                                         rhs=msg[:].rearrange("p h c -> p (h c)"),
                                         start=(ri == 0 and j == 0),
                                         stop=(ri == 1 and j == T - 1))
                # epilogue
                slw = sm_pool.tile([W_DST, C], F32, tag="slw")
                nc.sync.dma_start(slw[:], sl[t][ds(w * W_DST, W_DST), :])
                c1 = sm_pool.tile([W_DST, ow], F32, tag="c1")
                if li < 3:
                    nc.vector.tensor_add(c1[:], agg[:], eb_s[li, t][:])
                    nc.vector.tensor_tensor(
                        out=c1[:].rearrange("p (h c) -> p h c", h=H),
                        in0=c1[:].rearrange("p (h c) -> p h c", h=H),
                        in1=slw[:, None, :].to_broadcast([W_DST, H, C]),
                        op=mybir.AluOpType.add)
                else:
                    aggs = sm_pool.tile([W_DST, HC], F32, tag="aggs")
                    nc.scalar.copy(aggs[:], agg[:])
                    nc.vector.tensor_add(c1[:], aggs[:, 0:C], aggs[:, C:2 * C])
                    nc.vector.tensor_add(c1[:], c1[:], aggs[:, 2 * C:3 * C])
                    nc.vector.tensor_add(c1[:], c1[:], aggs[:, 3 * C:4 * C])
                    nc.scalar.mul(c1[:], c1[:], 0.25)
                    nc.vector.tensor_add(c1[:], c1[:], eb_s[li, t][:])
                    nc.vector.tensor_add(c1[:], c1[:], slw[:])
                neg = sm_pool.tile([W_DST, ow], F32, tag="neg")
                nc.vector.tensor_scalar(out=neg[:], in0=c1[:], scalar1=0.0,
                                        scalar2=None, op0=mybir.AluOpType.min)
                en = sm_pool.tile([W_DST, ow], F32, tag="en")
                nc.scalar.activation(en[:], neg[:], AF.Exp)
                pos = sm_pool.tile([W_DST, ow], F32, tag="pos")
                nc.vector.tensor_scalar(out=pos[:], in0=c1[:], scalar1=0.0,
                                        scalar2=None, op0=mybir.AluOpType.max)
                res = sm_pool.tile([W_DST, ow], F32, tag="res")
                nc.vector.tensor_add(res[:], pos[:], en[:])
                nc.scalar.activation(res[:], res[:], AF.Copy, bias=-1.0)
                nc.sync.dma_start(out_dram[ds(w * W_DST, W_DST), :], res[:])
            tc.For_i_unrolled(0, NW, 1, body, max_unroll=4)
            pb_ctx.__exit__(None, None, None); pa_ctx.__exit__(None, None, None)

        # ================= layers =================
        for li in (1, 2, 3):
            src = {1: {t: None for t in TYPES},
                   2: {t: xfull[t] for t in TYPES},
                   3: {t: xfull[t] for t in TYPES}}[li]
            srco = {1: {t: None for t in TYPES},
                    2: {t: shard[t] for t in TYPES},
                    3: {t: shard[t] for t in TYPES}}[li]
            for t in TYPES:
                node_full(li, t, src[t])
            for t in TYPES:
                node_own(li, t, srco[t])
            for t in TYPES:
                edge_phase(li, t)
            if li < 3:
                for t in TYPES:
                    nc.gpsimd.collective_compute(
                        "AllGather", mybir.AluOpType.bypass,
                        replica_groups=RG, ins=[shard[t][:]], outs=[xfull[t][:]])

        # ================= integrator =================
        pi_ctx = tc.tile_pool(name="pint", bufs=2, space="PSUM")
        ps_pool = pi_ctx.__enter__()
        def integ_body(i):
            xm = sm_pool.tile([W_DST, C], F32, tag="ixm")
            xi = sm_pool.tile([W_DST, C], F32, tag="ixi")
            nc.sync.dma_start(xm[:], x3['m'][ds(i * W_DST, W_DST), :])
            nc.sync.dma_start(xi[:], x3['i'][ds(i * W_DST, W_DST), :])
            sm_ = sm_pool.tile([W_DST, 1], F32, tag="ism")
            si_ = sm_pool.tile([W_DST, 1], F32, tag="isi")
            tmp = sm_pool.tile([W_DST, C], F32, tag="itmp")
            nc.vector.tensor_mul(tmp[:], xm[:], watt_s[:])
            nc.vector.reduce_sum(sm_[:], tmp[:], axis=mybir.AxisListType.X)
            nc.vector.tensor_mul(tmp[:], xi[:], watt_s[:])
            nc.vector.reduce_sum(si_[:], tmp[:], axis=mybir.AxisListType.X)
            dmi = sm_pool.tile([W_DST, 1], F32, tag="idm")
            nc.vector.tensor_sub(dmi[:], sm_[:], si_[:])
            am = sm_pool.tile([W_DST, 1], F32, tag="iam")
            ai = sm_pool.tile([W_DST, 1], F32, tag="iai")
            nc.scalar.activation(am[:], dmi[:], AF.Sigmoid)
            nc.vector.tensor_sub(dmi[:], si_[:], sm_[:])
            nc.scalar.activation(ai[:], dmi[:], AF.Sigmoid)
            fu = sm_pool.tile([W_DST, C], F32, tag="ifu")
            nc.vector.tensor_tensor(out=fu[:], in0=xm[:],
                                    in1=am[:].to_broadcast([W_DST, C]),
                                    op=mybir.AluOpType.mult)
            nc.vector.tensor_tensor(out=tmp[:], in0=xi[:],
                                    in1=ai[:].to_broadcast([W_DST, C]),
                                    op=mybir.AluOpType.mult)
            nc.vector.tensor_add(fu[:], fu[:], tmp[:])
            pt = ps_pool.tile([C, W_DST], F32, space="PSUM", tag="ipt")
            nc.tensor.transpose(pt[:], fu[:], idW)
            fT = sm_pool.tile([C, W_DST], F32, tag="ifT")
            nc.scalar.copy(fT[:], pt[:])
            hp = ps_pool.tile([W_DST, HID], F32, space="PSUM", tag="ihp")
            nc.tensor.matmul(hp[:], lhsT=fT[:], rhs=W1_s[:], start=True, stop=False)
            nc.tensor.matmul(hp[:], lhsT=ones1[:, 0:W_DST], rhs=b1_s[:],
                             start=False, stop=True)
            hs = sm_pool.tile([W_DST, HID], F32, tag="ihs")
            nc.scalar.activation(hs[:], hp[:], AF.Relu)
            pt2 = ps_pool.tile([HID, W_DST], F32, space="PSUM", tag="ipt2")
            nc.tensor.transpose(pt2[:], hs[:], idW)
            hT = sm_pool.tile([HID, W_DST], F32, tag="ihT")
            nc.scalar.copy(hT[:], pt2[:])
            op_ = ps_pool.tile([W_DST, CLASSES], F32, space="PSUM", tag="iop")
            nc.tensor.matmul(op_[:], lhsT=hT[:], rhs=W2_s[:], start=True, stop=False)
            nc.tensor.matmul(op_[:], lhsT=ones1[:, 0:W_DST], rhs=b2_s[:],
                             start=False, stop=True)
            os_ = sm_pool.tile([W_DST, CLASSES], F32, tag="ios")
            nc.scalar.copy(os_[:], op_[:])
            nc.sync.dma_start(out[ds(i * W_DST, W_DST), :], os_[:])
            os2 = sm_pool.tile([W_DST, CLASSES], F32, tag="ios2")
            nc.vector.tensor_copy(os2[:], os_[:])
            nc.sync.dma_start(out[ds(NLOC + i * W_DST, W_DST), :], os2[:])
        # NOTE: out rows [0:NLOC] = fused for m-type node ids, same values for
        # i-type ids because reference fuses types into one output per sample id
        tc.For_i_unrolled(0, NW, 1, integ_body, max_unroll=2)
        pi_ctx.__exit__(None, None, None)
        _es.close()

    nc.finalize()
    return nc


# ---------------------------------------------------------------- host side --
def _wrap_idxs(idx):
    n = idx.shape[0]
    w = idx.reshape(n // 16, 16).T.astype(np.int16)
    return np.tile(w, (8, 1))


def _prep_edges(edge, N, T_force=None):
    """edge [2, E] global. Returns per-core (gidx [NW,128,NI/16] int16,
    dcol [NW,128,T] f32) lists + T."""
    NLOC = N // NCORES
    NW = NLOC // W_DST
    src, dst = edge[0].astype(np.int64), edge[1].astype(np.int64)
    per_core = []
    maxT = 1
    for k in range(NCORES):
        m = (dst // NLOC) == k
        s, d = src[m], dst[m] - k * NLOC
        order = np.argsort(d, kind='stable')
        s, d = s[order], d[order]
        wins = []
        for w in range(NW):
            mm = (d // W_DST) == w
            sw, dw = s[mm], d[mm] % W_DST
            wins.append((sw, dw))
            maxT = max(maxT, (len(sw) + 127) // 128)
        per_core.append(wins)
    T = T_force or maxT
    NI = T * 128
    out = []
    for k in range(NCORES):
        gi = np.zeros((NW, 128, NI // 16), np.int16)
        dc = np.full((NW, 128, T), float(W_DST), np.float32)
        for w, (sw, dw) in enumerate(per_core[k]):
            n = len(sw)
            assert n <= NI, f"window overflow {n} > {NI}"
            si = np.zeros(NI, np.int64); si[:n] = sw
            gi[w] = _wrap_idxs(si)
            di = np.full(NI, float(W_DST), np.float32); di[:n] = dw
            dc[w] = di.reshape(T, 128).T
        out.append((gi.reshape(NW * 128, NI // 16), dc.reshape(NW * 128, T)))
    return out, T


def _np(x):
    return np.asarray(x, dtype=np.float32)


def kernel(x_mrna, x_mirna, params, edge_mm, edge_mi, edge_im, edge_ii,
           _N=None):
    N = _N or x_mrna.shape[0]
    NLOC = N // NCORES
    edges = {'mm': edge_mm, 'mi': edge_mi, 'im': edge_im, 'ii': edge_ii}
    prep = {}
    T = 1
    for r in RELS:
        prep[r], Tr = _prep_edges(np.asarray(edges[r]), N)
        T = max(T, Tr)
    # re-pad all to common T
    for r in RELS:
        prep[r], _ = _prep_edges(np.asarray(edges[r]), N, T_force=T)

    nc = build_program(N, T)

    x0 = {'m': _np(x_mrna), 'i': _np(x_mirna)}
    common = {}
    for t in TYPES:
        common[f"xT0_{t}"] = np.ascontiguousarray(x0[t].T)
    for li in (1, 2, 3):
        cp = params[f'conv{li}']
        slp = params[f'sl{li}']
        for r in RELS:
            p = cp[r]
            common[f"Wl{li}{r}"] = _np(p['Wl'])
            common[f"Wr{li}{r}"] = _np(p['Wr'])
            ab = np.zeros((HC, H), np.float32)
            att = _np(p['att'])
            for h in range(H):
                ab[h * C:(h + 1) * C, h] = att[h]
            common[f"att{li}{r}"] = ab
            common[f"brx{li}{r}"] = (_np(p['bl']) + _np(p['br']))[None, :]
        for t in TYPES:
            common[f"Wsl{li}{t}"] = _np(slp[t]['W'])
            rels_d = [r for r, (s, d) in RELS.items() if d == t]
            if li < 3:
                eb = sum(_np(cp[r]['bl']) + _np(cp[r]['bias']) for r in rels_d)
                eb = eb + np.tile(_np(slp[t]['b']), H)
            else:
                eb = sum(_np(cp[r]['bl']).reshape(H, C).mean(0) + _np(cp[r]['bias'])
                         for r in rels_d)
                eb = eb + _np(slp[t]['b'])
            common[f"eb{li}{t}"] = eb[None, :]
    ip = params['integ']
    common["watt"] = _np(ip['w_att'])[None, :]
    common["W1"] = _np(ip['W1']); common["b1"] = _np(ip['b1'])[None, :]
    common["W2"] = _np(ip['W2']); common["b2"] = _np(ip['b2'])[None, :]

    in_maps = []
    for k in range(NCORES):
        m = dict(common)
        for t in TYPES:
            m[f"xT0own_{t}"] = np.ascontiguousarray(
                x0[t][k * NLOC:(k + 1) * NLOC].T)
        for r in RELS:
            gi, dc = prep[r][k]
            m[f"gidx_{r}"] = gi
            m[f"dcol_{r}"] = dc
        in_maps.append(m)

    trace = bool(globals().get("TRACE"))
    if trace:
        _install_ntff_hook()
    res = run_bass_kernel_spmd(nc, in_maps, list(range(NCORES)), trace=trace)
    globals()["LAST_EXEC_NS"] = res.exec_time_ns
    # out rows per core: [NLOC m-fused, NLOC dup] -> reference output is per
    # sample id (types fused): take first NLOC rows of each core
    return np.concatenate([res.results[k]["out"][:NLOC] for k in range(NCORES)],
                          axis=0)


def _install_ntff_hook():
    import antenv
    if hasattr(antenv, "axon_hooks"):
        return
    from trn_agent_boot.trn_boot import _ntff_profile_via_ctypes
    hook = _ntff_profile_via_ctypes("/opt/axon/libaxon_pjrt.so")
    mod = types.ModuleType("antenv.axon_hooks")
    mod.get_axon_ntff_profile_hook = lambda: hook
    mod.set_axon_ntff_profile_hook = lambda h: None
    sys.modules["antenv.axon_hooks"] = mod
    antenv.axon_hooks = mod


# revision 13
# speedup vs baseline: 15.3951x; 13.2192x over previous
"""BiRGAT Trainium2 kernel: 8-core dst-sharded GNN message passing.

Strategy: shard edges by destination-node range (2500 dst rows per core, both
node types). Node-feature matmuls (xl tables) are replicated per core; per-edge
work is fully local (segment softmax + aggregation via one-hot sel matmuls on
sorted edges); layer outputs are AllGathered between layers. Gathers of
xl[src] rows use dma_gather (SWDGE) at window granularity.
"""
import sys, types
sys.path.insert(0, "/opt/trn_rl_repo")

import numpy as np
import ml_dtypes
import concourse.bass as bass
from concourse import bacc
import concourse.mybir as mybir
import concourse.tile as tile
from concourse.bass import ds
from concourse.bass_utils import run_bass_kernel_spmd
from concourse.masks import make_identity

F32 = mybir.dt.float32
AF = mybir.ActivationFunctionType

NCORES = 8
H, C = 4, 64
HC = H * C          # 256
PROJ = 256
CLASSES = 5
HID = 128
TYPES = ('m', 'i')
RELS = {'mm': ('m', 'm'), 'mi': ('m', 'i'), 'im': ('i', 'm'), 'ii': ('i', 'i')}
W_DST = 125          # dst rows per window


def build_program(N, T):
    """One SPMD program. N = nodes per type, T = edge tiles per window.
    Per core: NLOC = N//8 dst rows per type, NW = NLOC//W_DST windows."""
    NLOC = N // NCORES
    NW = NLOC // W_DST
    NI = T * 128                 # padded edges per window
    NTILE_FULL = N // W_DST      # full-table node tiles

    nc = bacc.Bacc(num_swdge_queues=2)

    # ---------------- I/O ----------------
    xT0 = {t: nc.dram_tensor(f"xT0_{t}", [PROJ, N], F32, kind="ExternalInput")
           for t in TYPES}
    xT0own = {t: nc.dram_tensor(f"xT0own_{t}", [PROJ, NLOC], F32, kind="ExternalInput")
              for t in TYPES}
    # per-relation edge data (per-core values)
    gidx = {r: nc.dram_tensor(f"gidx_{r}", [NW * 128, NI // 16], mybir.dt.int16,
                              kind="ExternalInput") for r in RELS}
    dcol = {r: nc.dram_tensor(f"dcol_{r}", [NW * 128, T], F32, kind="ExternalInput")
            for r in RELS}
    # weights per layer/relation
    Wl = {}; Wr = {}; attb = {}; brx = {}
    for li in (1, 2, 3):
        for r in RELS:
            Wl[li, r] = nc.dram_tensor(f"Wl{li}{r}", [HC, HC], F32, kind="ExternalInput")
            Wr[li, r] = nc.dram_tensor(f"Wr{li}{r}", [HC, HC], F32, kind="ExternalInput")
            attb[li, r] = nc.dram_tensor(f"att{li}{r}", [HC, H], F32, kind="ExternalInput")
            brx[li, r] = nc.dram_tensor(f"brx{li}{r}", [1, HC], F32, kind="ExternalInput")
    Wsl = {}; ebias = {}
    for li in (1, 2, 3):
        for t in TYPES:
            Wsl[li, t] = nc.dram_tensor(f"Wsl{li}{t}", [HC, C], F32, kind="ExternalInput")
            eb_w = HC if li < 3 else C
            ebias[li, t] = nc.dram_tensor(f"eb{li}{t}", [1, eb_w], F32, kind="ExternalInput")
    watt = nc.dram_tensor("watt", [1, C], F32, kind="ExternalInput")
    W1 = nc.dram_tensor("W1", [C, HID], F32, kind="ExternalInput")
    b1 = nc.dram_tensor("b1", [1, HID], F32, kind="ExternalInput")
    W2 = nc.dram_tensor("W2", [HID, CLASSES], F32, kind="ExternalInput")
    b2 = nc.dram_tensor("b2", [1, CLASSES], F32, kind="ExternalInput")
    out = nc.dram_tensor("out", [NLOC * 2, CLASSES], F32, kind="ExternalOutput")

    # ---------------- internal DRAM ----------------
    xl = {r: nc.dram_tensor(f"xl_{r}", [N, HC], F32) for r in RELS}       # per layer reuse
    xr = {r: nc.dram_tensor(f"xr_{r}", [NLOC, HC], F32) for r in RELS}
    sl = {t: nc.dram_tensor(f"sl_{t}", [NLOC, C], F32) for t in TYPES}
    shard = {t: nc.dram_tensor(f"shard_{t}", [NLOC, HC], F32) for t in TYPES}
    xfull = {t: nc.dram_tensor(f"xfull_{t}", [N, HC], F32, addr_space="Shared")
             for t in TYPES}
    x3 = {t: nc.dram_tensor(f"x3_{t}", [NLOC, C], F32) for t in TYPES}

    rels_by_src = {t: [r for r, (s, d) in RELS.items() if s == t] for t in TYPES}
    rels_by_dst = {t: [r for r, (s, d) in RELS.items() if d == t] for t in TYPES}
    RG = [list(range(NCORES))]

    from contextlib import ExitStack
    _es = ExitStack()
    with tile.TileContext(nc) as tc:
        cpool = _es.enter_context(tc.tile_pool(name="const", bufs=1))
        ident = cpool.tile([128, 128], F32)
        make_identity(nc, ident[:])
        iota_i = cpool.tile([128, W_DST], mybir.dt.int32)
        nc.gpsimd.iota(iota_i[:], pattern=[[1, W_DST]], base=0, channel_multiplier=0)
        iota = cpool.tile([128, W_DST], F32)
        nc.vector.tensor_copy(iota[:], iota_i[:])
        idW = ident[0:W_DST, 0:W_DST]
        ones1 = cpool.tile([1, 128], F32)
        nc.vector.memset(ones1[:], 1.0)

        # weight tiles resident in SBUF
        wpool = _es.enter_context(tc.tile_pool(name="weights", bufs=1))
        Wl_s = {}; Wr_s = {}; attb_s = {}; brx_s = {}; Wsl_s = {}; eb_s = {}
        for li in (1, 2, 3):
            for r in RELS:
                for si, (dram, store) in enumerate(((Wl[li, r], Wl_s), (Wr[li, r], Wr_s))):
                    tl = wpool.tile([128, 2, HC], F32, tag=f"w{li}{r}{si}")
                    nc.sync.dma_start(tl[:, 0, :], dram[0:128, :])
                    nc.sync.dma_start(tl[:, 1, :], dram[128:256, :])
                    store[li, r] = tl
                a = wpool.tile([128, 2, H], F32, tag=f"a{li}{r}")
                nc.sync.dma_start(a[:, 0, :], attb[li, r][0:128, :])
                nc.sync.dma_start(a[:, 1, :], attb[li, r][128:256, :])
                attb_s[li, r] = a
                b = wpool.tile([1, HC], F32, tag=f"b{li}{r}")
                nc.sync.dma_start(b[:], brx[li, r][:])
                brx_s[li, r] = b
            for t in TYPES:
                wt = wpool.tile([128, 2, C], F32, tag=f"wsl{li}{t}")
                nc.sync.dma_start(wt[:, 0, :], Wsl[li, t][0:128, :])
                nc.sync.dma_start(wt[:, 1, :], Wsl[li, t][128:256, :])
                Wsl_s[li, t] = wt
                eb_w = HC if li < 3 else C
                e = wpool.tile([W_DST, eb_w], F32, tag=f"eb{li}{t}")
                nc.sync.dma_start(e[:], ebias[li, t][:].to_broadcast([W_DST, eb_w]))
                eb_s[li, t] = e
        watt_s = wpool.tile([W_DST, C], F32)
        nc.sync.dma_start(watt_s[:], watt[:].to_broadcast([W_DST, C]))
        W1_s = wpool.tile([C, HID], F32); nc.sync.dma_start(W1_s[:], W1[:])
        b1_s = wpool.tile([1, HID], F32); nc.sync.dma_start(b1_s[:], b1[:])
        W2_s = wpool.tile([HID, CLASSES], F32); nc.sync.dma_start(W2_s[:], W2[:])
        b2_s = wpool.tile([1, CLASSES], F32); nc.sync.dma_start(b2_s[:], b2[:])

        np_pool = _es.enter_context(tc.tile_pool(name="node", bufs=3))
        ed_pool = _es.enter_context(tc.tile_pool(name="edge", bufs=3))
        sm_pool = _es.enter_context(tc.tile_pool(name="small", bufs=4))

        def node_full(li, t, src_dram):
            """xl tables for rels with src type t over all N rows.
            src_dram: None for layer1 (use xT0), else xfull[t]."""
            ps_ctx = tc.tile_pool(name="psn", bufs=2, space="PSUM")
            ps_pool = ps_ctx.__enter__()
            def body(i):
                if src_dram is None:
                    l0 = np_pool.tile([128, W_DST], F32, tag="l0")
                    l1 = np_pool.tile([128, W_DST], F32, tag="l1")
                    nc.sync.dma_start(l0[:], xT0[t][0:128, ds(i * W_DST, W_DST)])
                    nc.sync.dma_start(l1[:], xT0[t][128:256, ds(i * W_DST, W_DST)])
                else:
                    xin = np_pool.tile([W_DST, HC], F32, tag="xin")
                    nc.sync.dma_start(xin[:], src_dram[ds(i * W_DST, W_DST), :])
                    l0 = np_pool.tile([128, W_DST], F32, tag="l0")
                    l1 = np_pool.tile([128, W_DST], F32, tag="l1")
                    for h, lt in ((0, l0), (1, l1)):
                        pt = ps_pool.tile([128, W_DST], F32, space="PSUM", tag="ptr")
                        nc.tensor.transpose(pt[:], xin[:, h * 128:(h + 1) * 128], idW)
                        nc.scalar.copy(lt[:], pt[:])
                for r in rels_by_src[t]:
                    ps = ps_pool.tile([W_DST, HC], F32, space="PSUM", tag="pnode")
                    nc.tensor.matmul(ps[:], lhsT=l0[:], rhs=Wl_s[li, r][:, 0, :],
                                     start=True, stop=False)
                    nc.tensor.matmul(ps[:], lhsT=l1[:], rhs=Wl_s[li, r][:, 1, :],
                                     start=False, stop=True)
                    xs = np_pool.tile([W_DST, HC], F32, tag="xs")
                    nc.scalar.copy(xs[:], ps[:])
                    nc.sync.dma_start(xl[r][ds(i * W_DST, W_DST), :], xs[:])
            tc.For_i_unrolled(0, NTILE_FULL, 1, body, max_unroll=8)
            ps_ctx.__exit__(None, None, None)

        def node_own(li, t, src_dram):
            """xr tables (rels with dst type t) + selfloop, own rows only.
            src_dram: None for layer1 (xT0own), else shard[t] (local copy)."""
            ps_ctx = tc.tile_pool(name="pso", bufs=2, space="PSUM")
            ps_pool = ps_ctx.__enter__()
            def body(i):
                if src_dram is None:
                    l0 = np_pool.tile([128, W_DST], F32, tag="l0")
                    l1 = np_pool.tile([128, W_DST], F32, tag="l1")
                    nc.sync.dma_start(l0[:], xT0own[t][0:128, ds(i * W_DST, W_DST)])
                    nc.sync.dma_start(l1[:], xT0own[t][128:256, ds(i * W_DST, W_DST)])
                else:
                    xin = np_pool.tile([W_DST, HC], F32, tag="xin")
                    nc.sync.dma_start(xin[:], src_dram[ds(i * W_DST, W_DST), :])
                    l0 = np_pool.tile([128, W_DST], F32, tag="l0")
                    l1 = np_pool.tile([128, W_DST], F32, tag="l1")
                    for h, lt in ((0, l0), (1, l1)):
                        pt = ps_pool.tile([128, W_DST], F32, space="PSUM", tag="ptr")
                        nc.tensor.transpose(pt[:], xin[:, h * 128:(h + 1) * 128], idW)
                        nc.scalar.copy(lt[:], pt[:])
                for r in rels_by_dst[t]:
                    ps = ps_pool.tile([W_DST, HC], F32, space="PSUM", tag="pnode")
                    nc.tensor.matmul(ps[:], lhsT=l0[:], rhs=Wr_s[li, r][:, 0, :],
                                     start=True, stop=False)
                    nc.tensor.matmul(ps[:], lhsT=l1[:], rhs=Wr_s[li, r][:, 1, :],
                                     start=False, stop=False)
                    nc.tensor.matmul(ps[:], lhsT=ones1[:, 0:W_DST], rhs=brx_s[li, r][:],
                                     start=False, stop=True)
                    xs = np_pool.tile([W_DST, HC], F32, tag="xs")
                    nc.scalar.copy(xs[:], ps[:])
                    nc.sync.dma_start(xr[r][ds(i * W_DST, W_DST), :], xs[:])
                ps = ps_pool.tile([W_DST, C], F32, space="PSUM", tag="psl")
                nc.tensor.matmul(ps[:], lhsT=l0[:], rhs=Wsl_s[li, t][:, 0, :],
                                 start=True, stop=False)
                nc.tensor.matmul(ps[:], lhsT=l1[:], rhs=Wsl_s[li, t][:, 1, :],
                                 start=False, stop=True)
                xs = np_pool.tile([W_DST, C], F32, tag="xsl")
                nc.scalar.copy(xs[:], ps[:])
                nc.sync.dma_start(sl[t][ds(i * W_DST, W_DST), :], xs[:])
            tc.For_i_unrolled(0, NW, 1, body, max_unroll=4)
            ps_ctx.__exit__(None, None, None)

        def edge_phase(li, t):
            """For each window: two relations -> agg psum -> epilogue."""
            relA, relB = rels_by_dst[t]
            out_dram = shard[t] if li < 3 else x3[t]
            ow = HC if li < 3 else C
            pa_ctx = tc.tile_pool(name="pea", bufs=1, space="PSUM")
            pb_ctx = tc.tile_pool(name="peb", bufs=3, space="PSUM")
            pa = pa_ctx.__enter__(); pb = pb_ctx.__enter__()

            def body(w):
                agg = pa.tile([W_DST, HC], F32, space="PSUM", tag="agg")
                for ri, r in enumerate((relA, relB)):
                    it = sm_pool.tile([128, NI // 16], mybir.dt.int16, tag="it")
                    nc.sync.dma_start(it[:], gidx[r][ds(w * 128, 128), :])
                    gb = ed_pool.tile([128, T, HC], F32, tag="gb")
                    nc.gpsimd.dma_gather(out_ap=gb[:], in_ap=xl[r][:], idxs_ap=it[:],
                                         num_idxs=NI, num_idxs_reg=NI, elem_size=HC,
                                         queue_num=ri, single_packet=False)
                    dc = sm_pool.tile([128, T], F32, tag="dc")
                    nc.sync.dma_start(dc[:], dcol[r][ds(w * 128, 128), :])
                    xrw = sm_pool.tile([W_DST, HC], F32, tag="xrw")
                    nc.sync.dma_start(xrw[:], xr[r][ds(w * W_DST, W_DST), :])
                    selb = ed_pool.tile([128, T, W_DST], F32, tag="selb")
                    selTb = ed_pool.tile([W_DST, T, 128], F32, tag="selTb")
                    expb = ed_pool.tile([128, T, H], F32, tag="expb")
                    den = pa.tile([W_DST, H], F32, space="PSUM", tag="den")
                    for j in range(T):
                        nc.vector.tensor_tensor(
                            out=selb[:, j, :],
                            in0=dc[:, j:j + 1].to_broadcast([128, W_DST]),
                            in1=iota[:], op=mybir.AluOpType.is_equal)
                        pt = pa.tile([W_DST, 128], F32, space="PSUM", tag="pselT")
                        nc.tensor.transpose(pt[:], selb[:, j, :], ident[:])
                        nc.vector.tensor_copy(selTb[:, j, :], pt[:])
                        zt = pb.tile([128, 2, 128], F32, space="PSUM", tag="zt")
                        wt = ed_pool.tile([128, 2, 128], F32, tag="wt")
                        sc = pa.tile([128, H], F32, space="PSUM", tag="sc")
                        for h in range(2):
                            nc.tensor.matmul(zt[:, h, :], lhsT=gb[:, j, ds(h * 128, 128)],
                                             rhs=ident[:], is_transpose=True,
                                             start=True, stop=False)
                            nc.tensor.matmul(zt[:, h, :], lhsT=xrw[:, ds(h * 128, 128)],
                                             rhs=selTb[:, j, :], start=False, stop=True)
                            nc.scalar.activation(wt[:, h, :], zt[:, h, :], AF.Prelu,
                                                 alpha=0.2)
                            nc.tensor.matmul(sc[:], lhsT=wt[:, h, :],
                                             rhs=attb_s[li, r][:, h, :],
                                             start=(h == 0), stop=(h == 1))
                        nc.scalar.activation(expb[:, j, :], sc[:], AF.Exp)
                        nc.tensor.matmul(den[:], lhsT=selb[:, j, :], rhs=expb[:, j, :],
                                         start=(j == 0), stop=(j == T - 1))
                    rden = sm_pool.tile([W_DST, H], F32, tag="rden")
                    dent = sm_pool.tile([W_DST, H], F32, tag="dent")
                    nc.scalar.activation(dent[:], den[:], AF.Copy, bias=1e-16)
                    nc.vector.reciprocal(rden[:], dent[:])
                    for j in range(T):
                        rex = pa.tile([128, H], F32, space="PSUM", tag="rex")
                        nc.tensor.matmul(rex[:], lhsT=selTb[:, j, :], rhs=rden[:],
                                         start=True, stop=True)
                        alp = sm_pool.tile([128, H], F32, tag="alp")
                        nc.vector.tensor_mul(alp[:], expb[:, j, :], rex[:])
                        msg = ed_pool.tile([128, H, C], F32, tag="msg")
                        nc.vector.tensor_tensor(
                            out=msg[:],
                            in0=gb[:, j, :].rearrange("p (h c) -> p h c", h=H),
                            in1=alp[:, :, None].to_broadcast([128, H, C]),
                            op=mybir.AluOpType.mult)
                        nc.tensor.matmul(agg[:], lhsT=selb[:, j, :],
                                         rhs=msg[:].rearrange("p h c -> p (h c)"),
                                         start=(ri == 0 and j == 0),
                                         stop=(ri == 1 and j == T - 1))
                # epilogue
                slw = sm_pool.tile([W_DST, C], F32, tag="slw")
                nc.sync.dma_start(slw[:], sl[t][ds(w * W_DST, W_DST), :])
                c1 = sm_pool.tile([W_DST, ow], F32, tag="c1")
                if li < 3:
                    nc.vector.tensor_add(c1[:], agg[:], eb_s[li, t][:])
                    nc.vector.tensor_tensor(
                        out=c1[:].rearrange("p (h c) -> p h c", h=H),
                        in0=c1[:].rearrange("p (h c) -> p h c", h=H),
                        in1=slw[:, None, :].to_broadcast([W_DST, H, C]),
                        op=mybir.AluOpType.add)
                else:
                    aggs = sm_pool.tile([W_DST, HC], F32, tag="aggs")
                    nc.scalar.copy(aggs[:], agg[:])
                    nc.vector.tensor_add(c1[:], aggs[:, 0:C], aggs[:, C:2 * C])
                    nc.vector.tensor_add(c1[:], c1[:], aggs[:, 2 * C:3 * C])
                    nc.vector.tensor_add(c1[:], c1[:], aggs[:, 3 * C:4 * C])
                    nc.scalar.mul(c1[:], c1[:], 0.25)
                    nc.vector.tensor_add(c1[:], c1[:], eb_s[li, t][:])
                    nc.vector.tensor_add(c1[:], c1[:], slw[:])
                neg = sm_pool.tile([W_DST, ow], F32, tag="neg")
                nc.vector.tensor_scalar(out=neg[:], in0=c1[:], scalar1=0.0,
                                        scalar2=None, op0=mybir.AluOpType.min)
                en = sm_pool.tile([W_DST, ow], F32, tag="en")
                nc.scalar.activation(en[:], neg[:], AF.Exp)
                pos = sm_pool.tile([W_DST, ow], F32, tag="pos")
                nc.vector.tensor_scalar(out=pos[:], in0=c1[:], scalar1=0.0,
                                        scalar2=None, op0=mybir.AluOpType.max)
                res = sm_pool.tile([W_DST, ow], F32, tag="res")
                nc.vector.tensor_add(res[:], pos[:], en[:])
                nc.scalar.activation(res[:], res[:], AF.Copy, bias=-1.0)
                nc.sync.dma_start(out_dram[ds(w * W_DST, W_DST), :], res[:])
            tc.For_i_unrolled(0, NW, 1, body, max_unroll=4)
            pb_ctx.__exit__(None, None, None); pa_ctx.__exit__(None, None, None)

        # ================= layers =================
        for li in (1, 2, 3):
            src = {1: {t: None for t in TYPES},
                   2: {t: xfull[t] for t in TYPES},
                   3: {t: xfull[t] for t in TYPES}}[li]
            srco = {1: {t: None for t in TYPES},
                    2: {t: shard[t] for t in TYPES},
                    3: {t: shard[t] for t in TYPES}}[li]
            for t in TYPES:
                node_full(li, t, src[t])
            for t in TYPES:
                node_own(li, t, srco[t])
            for t in TYPES:
                edge_phase(li, t)
            if li < 3:
                for t in TYPES:
                    nc.gpsimd.collective_compute(
                        "AllGather", mybir.AluOpType.bypass,
                        replica_groups=RG, ins=[shard[t][:]], outs=[xfull[t][:]])

        # ================= integrator =================
        pi_ctx = tc.tile_pool(name="pint", bufs=2, space="PSUM")
        ps_pool = pi_ctx.__enter__()
        def integ_body(i):
            xm = sm_pool.tile([W_DST, C], F32, tag="ixm")
            xi = sm_pool.tile([W_DST, C], F32, tag="ixi")
            nc.sync.dma_start(xm[:], x3['m'][ds(i * W_DST, W_DST), :])
            nc.sync.dma_start(xi[:], x3['i'][ds(i * W_DST, W_DST), :])
            sm_ = sm_pool.tile([W_DST, 1], F32, tag="ism")
            si_ = sm_pool.tile([W_DST, 1], F32, tag="isi")
            tmp = sm_pool.tile([W_DST, C], F32, tag="itmp")
            nc.vector.tensor_mul(tmp[:], xm[:], watt_s[:])
            nc.vector.reduce_sum(sm_[:], tmp[:], axis=mybir.AxisListType.X)
            nc.vector.tensor_mul(tmp[:], xi[:], watt_s[:])
            nc.vector.reduce_sum(si_[:], tmp[:], axis=mybir.AxisListType.X)
            dmi = sm_pool.tile([W_DST, 1], F32, tag="idm")
            nc.vector.tensor_sub(dmi[:], sm_[:], si_[:])
            am = sm_pool.tile([W_DST, 1], F32, tag="iam")
            ai = sm_pool.tile([W_DST, 1], F32, tag="iai")
            nc.scalar.activation(am[:], dmi[:], AF.Sigmoid)
            nc.vector.tensor_sub(dmi[:], si_[:], sm_[:])
            nc.scalar.activation(ai[:], dmi[:], AF.Sigmoid)
            fu = sm_pool.tile([W_DST, C], F32, tag="ifu")
            nc.vector.tensor_tensor(out=fu[:], in0=xm[:],
                                    in1=am[:].to_broadcast([W_DST, C]),
                                    op=mybir.AluOpType.mult)
            nc.vector.tensor_tensor(out=tmp[:], in0=xi[:],
                                    in1=ai[:].to_broadcast([W_DST, C]),
                                    op=mybir.AluOpType.mult)
            nc.vector.tensor_add(fu[:], fu[:], tmp[:])
            pt = ps_pool.tile([C, W_DST], F32, space="PSUM", tag="ipt")
            nc.tensor.transpose(pt[:], fu[:], idW)
            fT = sm_pool.tile([C, W_DST], F32, tag="ifT")
            nc.scalar.copy(fT[:], pt[:])
            hp = ps_pool.tile([W_DST, HID], F32, space="PSUM", tag="ihp")
            nc.tensor.matmul(hp[:], lhsT=fT[:], rhs=W1_s[:], start=True, stop=False)
            nc.tensor.matmul(hp[:], lhsT=ones1[:, 0:W_DST], rhs=b1_s[:],
                             start=False, stop=True)
            hs = sm_pool.tile([W_DST, HID], F32, tag="ihs")
            nc.scalar.activation(hs[:], hp[:], AF.Relu)
            pt2 = ps_pool.tile([HID, W_DST], F32, space="PSUM", tag="ipt2")
            nc.tensor.transpose(pt2[:], hs[:], idW)
            hT = sm_pool.tile([HID, W_DST], F32, tag="ihT")
            nc.scalar.copy(hT[:], pt2[:])
            op_ = ps_pool.tile([W_DST, CLASSES], F32, space="PSUM", tag="iop")
            nc.tensor.matmul(op_[:], lhsT=hT[:], rhs=W2_s[:], start=True, stop=False)
            nc.tensor.matmul(op_[:], lhsT=ones1[:, 0:W_DST], rhs=b2_s[:],
                             start=False, stop=True)
            os_ = sm_pool.tile([W_DST, CLASSES], F32, tag="ios")
            nc.scalar.copy(os_[:], op_[:])
            nc.sync.dma_start(out[ds(i * W_DST, W_DST), :], os_[:])
            os2 = sm_pool.tile([W_DST, CLASSES], F32, tag="ios2")
            nc.vector.tensor_copy(os2[:], os_[:])
            nc.sync.dma_start(out[ds(NLOC + i * W_DST, W_DST), :], os2[:])
        # NOTE: out rows [0:NLOC] = fused for m-type node ids, same values for
        # i-type ids because reference fuses types into one output per sample id
        tc.For_i_unrolled(0, NW, 1, integ_body, max_unroll=2)
        pi_ctx.__exit__(None, None, None)
        _es.close()

    nc.finalize()
    return nc


# ---------------------------------------------------------------- host side --
def _wrap_idxs(idx):
    n = idx.shape[0]
    w = idx.reshape(n // 16, 16).T.astype(np.int16)
    return np.tile(w, (8, 1))


def _prep_edges(edge, N, T_force=None):
    """edge [2, E] global. Returns per-core (gidx [NW,128,NI/16] int16,
    dcol [NW,128,T] f32) lists + T."""
    NLOC = N // NCORES
    NW = NLOC // W_DST
    src, dst = edge[0].astype(np.int64), edge[1].astype(np.int64)
    per_core = []
    maxT = 1
    for k in range(NCORES):
        m = (dst // NLOC) == k
        s, d = src[m], dst[m] - k * NLOC
        order = np.argsort(d, kind='stable')
        s, d = s[order], d[order]
        wins = []
        for w in range(NW):
            mm = (d // W_DST) == w
            sw, dw = s[mm], d[mm] % W_DST
            wins.append((sw, dw))
            maxT = max(maxT, (len(sw) + 127) // 128)
        per_core.append(wins)
    T = T_force or maxT
    NI = T * 128
    out = []
    for k in range(NCORES):
        gi = np.zeros((NW, 128, NI // 16), np.int16)
        dc = np.full((NW, 128, T), float(W_DST), np.float32)
        for w, (sw, dw) in enumerate(per_core[k]):
            n = len(sw)
            assert n <= NI, f"window overflow {n} > {NI}"
            si = np.zeros(NI, np.int64); si[:n] = sw
            gi[w] = _wrap_idxs(si)
            di = np.full(NI, float(W_DST), np.float32); di[:n] = dw
            dc[w] = di.reshape(T, 128).T
        out.append((gi.reshape(NW * 128, NI // 16), dc.reshape(NW * 128, T)))
    return out, T


def _np(x):
    return np.asarray(x, dtype=np.float32)


def kernel(x_mrna, x_mirna, params, edge_mm, edge_mi, edge_im, edge_ii,
           _N=None):
    N = _N or x_mrna.shape[0]
    NLOC = N // NCORES
    edges = {'mm': edge_mm, 'mi': edge_mi, 'im': edge_im, 'ii': edge_ii}
    prep = {}
    T = 1
    for r in RELS:
        prep[r], Tr = _prep_edges(np.asarray(edges[r]), N)
        T = max(T, Tr)
    # re-pad all to common T
    for r in RELS:
        prep[r], _ = _prep_edges(np.asarray(edges[r]), N, T_force=T)

    nc = build_program(N, T)

    x0 = {'m': _np(x_mrna), 'i': _np(x_mirna)}
    common = {}
    for t in TYPES:
        common[f"xT0_{t}"] = np.ascontiguousarray(x0[t].T)
    for li in (1, 2, 3):
        cp = params[f'conv{li}']
        slp = params[f'sl{li}']
        for r in RELS:
            p = cp[r]
            common[f"Wl{li}{r}"] = _np(p['Wl'])
            common[f"Wr{li}{r}"] = _np(p['Wr'])
            ab = np.zeros((HC, H), np.float32)
            att = _np(p['att'])
            for h in range(H):
                ab[h * C:(h + 1) * C, h] = att[h]
            common[f"att{li}{r}"] = ab
            common[f"brx{li}{r}"] = (_np(p['bl']) + _np(p['br']))[None, :]
        for t in TYPES:
            common[f"Wsl{li}{t}"] = _np(slp[t]['W'])
            rels_d = [r for r, (s, d) in RELS.items() if d == t]
            if li < 3:
                eb = sum(_np(cp[r]['bl']) + _np(cp[r]['bias']) for r in rels_d)
                eb = eb + np.tile(_np(slp[t]['b']), H)
            else:
                eb = sum(_np(cp[r]['bl']).reshape(H, C).mean(0) + _np(cp[r]['bias'])
                         for r in rels_d)
                eb = eb + _np(slp[t]['b'])
            common[f"eb{li}{t}"] = eb[None, :]
    ip = params['integ']
    common["watt"] = _np(ip['w_att'])[None, :]
    common["W1"] = _np(ip['W1']); common["b1"] = _np(ip['b1'])[None, :]
    common["W2"] = _np(ip['W2']); common["b2"] = _np(ip['b2'])[None, :]

    in_maps = []
    for k in range(NCORES):
        m = dict(common)
        for t in TYPES:
            m[f"xT0own_{t}"] = np.ascontiguousarray(
                x0[t][k * NLOC:(k + 1) * NLOC].T)
        for r in RELS:
            gi, dc = prep[r][k]
            m[f"gidx_{r}"] = gi
            m[f"dcol_{r}"] = dc
        in_maps.append(m)

    trace = bool(globals().get("TRACE"))
    if trace:
        _install_ntff_hook()
    res = run_bass_kernel_spmd(nc, in_maps, list(range(NCORES)), trace=trace)
    globals()["LAST_EXEC_NS"] = res.exec_time_ns
    # out rows per core: [NLOC m-fused, NLOC dup] -> reference output is per
    # sample id (types fused): take first NLOC rows of each core
    return np.concatenate([res.results[k]["out"][:NLOC] for k in range(NCORES)],
                          axis=0)


def _install_ntff_hook():
    import antenv
    if hasattr(antenv, "axon_hooks"):
        return
    from trn_agent_boot.trn_boot import _ntff_profile_via_ctypes
    hook = _ntff_profile_via_ctypes("/opt/axon/libaxon_pjrt.so")
    mod = types.ModuleType("antenv.axon_hooks")
    mod.get_axon_ntff_profile_hook = lambda: hook
    mod.set_axon_ntff_profile_hook = lambda h: None
    sys.modules["antenv.axon_hooks"] = mod
    antenv.axon_hooks = mod
